# revision 1
# baseline (speedup 1.0000x reference)
"""MoE routing kernel for Trainium2 (8 NeuronCores, expert-parallel).

Model (per layer l in 0..L-1):
    w = softmax(top-k masked |x @ protos[l].T|)          # [N, E] routing
    h = relu(x @ W1[l,e]); y = sum_e w[:,e] * (h @ W2[l,e])
    x = x + y

Sharding: expert-parallel - core c owns expert c for both layers (E == 8 ==
n_cores).  Every core computes the routing for all tokens (cheap), runs its
expert's FFN over all tokens in a feature-major ("transposed") layout so the
weights load untransposed as the stationary matmul operand, scales by its
routing column, and the weighted partials are summed with an on-device
AllReduce.  Core 0 additionally folds the residual x into its partial, so the
AllReduce output IS the next layer's input.  A second AllReduce after layer 2
produces the final output on every core.

The kernel is built once and cached at module level; repeated kernel() calls
reuse the compiled executable.
"""

import numpy as np

import concourse.bacc as bacc
import concourse.mybir as mybir
from concourse import tile
from concourse.masks import make_identity

P = 128


def build_moe(
    nc,
    D=1024,
    F=2048,
    NTOK=2048,
    E=8,
    L=2,
    K=2,
    BLK=256,
    ffn_f32r=True,
    nsplit=4,
):
    """Emit the SPMD MoE program into Bass instance `nc`."""
    DS = D // P       # D-slices (k-tiles for W1 / m-tiles for W2 output)
    FS = F // P       # F-slices
    NBLK = NTOK // BLK
    TT = BLK // P     # token-tiles per block
    f32 = mybir.dt.float32
    f32r = mybir.dt.float32r
    # fp32r operands must be explicitly rounded by their producer; weights are
    # rounded in place after load (bitcast view), x gets a rounded copy, and
    # the relu writes f32r directly.
    ffd = f32r if ffn_f32r else f32

    xT = nc.dram_tensor("xT", [D, NTOK], f32, kind="ExternalInput")
    prot = nc.dram_tensor("prot", [L, D, E], ffd, kind="ExternalInput")
    w1 = nc.dram_tensor("w1", [L, D, F], ffd, kind="ExternalInput")
    w2 = nc.dram_tensor("w2", [L, F, D], ffd, kind="ExternalInput")
    alpha = nc.dram_tensor("alpha", [P, 1], f32, kind="ExternalInput")
    onehot = nc.dram_tensor("onehot", [P, E], f32, kind="ExternalInput")
    yout = nc.dram_tensor("yout", [D, NTOK], f32, kind="ExternalOutput")

    with tile.TileContext(nc) as tc:
        with (
            tc.tile_pool(name="const", bufs=1) as constp,
            tc.tile_pool(name="wpool", bufs=1) as wpool,
            tc.tile_pool(name="xpool", bufs=2) as xpool,
            tc.tile_pool(name="route", bufs=2) as routep,
            tc.tile_pool(name="hpool", bufs=1) as hpool,
            tc.tile_pool(name="evict", bufs=3) as evpool,
            tc.tile_pool(name="wbcp", bufs=2) as wbcp,
            tc.tile_pool(name="psmisc", bufs=2, space="PSUM") as psmisc,
            tc.tile_pool(name="psh", bufs=2, space="PSUM") as psh,
            tc.tile_pool(name="psy", bufs=4, space="PSUM") as psy,
            tc.tile_pool(name="dram", bufs=2, space="DRAM") as dramp,
        ):
            ident = constp.tile([P, P], f32)
            make_identity(nc, ident)
            ones_row = constp.tile([1, P], f32)
            nc.vector.memset(ones_row, 1.0)
            alpha_sb = constp.tile([P, 1], f32)
            nc.sync.dma_start(alpha_sb[:], alpha.ap()[:])
            oh_sb = constp.tile([P, E], f32)
            nc.sync.dma_start(oh_sb[:], onehot.ap()[:])

            NH = NTOK // nsplit  # tokens per AR slice
            HBLK = NH // BLK     # blocks per slice
            assert HBLK >= 1
            xsrc_halves = [xT.ap()[:, h * NH : (h + 1) * NH] for h in range(nsplit)]
            for l in range(L):
                ypart = [
                    dramp.tile([D, NH], f32, tag=f"ypart{h}", name=f"ypart{h}_{l}")
                    for h in range(nsplit)
                ]
                ysum = [
                    dramp.tile([D, NH], f32, tag=f"ysum{h}", name=f"ysum{h}_{l}")
                    for h in range(nsplit)
                ]

                prot_sb = wpool.tile([P, DS, E], ffd, tag="prot")
                nc.sync.dma_start(
                    prot_sb[:], prot.ap()[l].rearrange("(t p) e -> p t e", p=P)
                )
                w1_sb = wpool.tile([P, DS, F], ffd, tag="w1")
                for ds in range(DS):
                    nc.sync.dma_start(
                        w1_sb[:, ds, :], w1.ap()[l, ds * P : (ds + 1) * P, :]
                    )
                w2_sb = wpool.tile([P, FS, D], ffd, tag="w2")
                for fs in range(FS):
                    nc.sync.dma_start(
                        w2_sb[:, fs, :], w2.ap()[l, fs * P : (fs + 1) * P, :]
                    )

                for nb in range(NBLK):
                    half = nb // HBLK
                    c0 = nb * BLK - half * NH  # col offset within the half
                    xsrc = xsrc_halves[half]
                    xb = xpool.tile([P, DS, BLK], f32, tag="xb")
                    for ds in range(DS):
                        nc.sync.dma_start(
                            xb[:, ds, :], xsrc[ds * P : (ds + 1) * P, c0 : c0 + BLK]
                        )
                    if ffn_f32r:
                        xbr = xpool.tile([P, DS, BLK], f32r, tag="xbr")
                        nc.vector.tensor_copy(xbr[:], xb[:])
                    else:
                        xbr = xb

                    # ---- routing: w column for this core's expert ----
                    ps_s = psmisc.tile([E, BLK], f32, tag="psm")
                    for ds in range(DS):
                        nc.tensor.matmul(
                            ps_s[:],
                            prot_sb[:, ds, :],
                            xbr[:, ds, :],
                            start=(ds == 0),
                            stop=(ds == DS - 1),
                        )
                    s_abs = routep.tile([E, BLK], f32, tag="sabs")
                    nc.scalar.activation(
                        s_abs[:], ps_s[:], mybir.ActivationFunctionType.Abs
                    )
                    s_tok = routep.tile([P, TT, E], f32, tag="stok")
                    for tt in range(TT):
                        ps_t = psmisc.tile([P, E], f32, tag="psm")
                        nc.tensor.transpose(
                            ps_t[:], s_abs[:, tt * P : (tt + 1) * P], ident[:E, :E]
                        )
                        nc.scalar.copy(s_tok[:, tt, :], ps_t[:])
                    srt = routep.tile([P, TT, E], f32, tag="srt")
                    for tt in range(TT):
                        nc.vector.max(srt[:, tt, :], s_tok[:, tt, :])
                    shif = routep.tile([P, TT, E], f32, tag="shif")
                    nc.vector.tensor_tensor(
                        out=shif[:],
                        in0=s_tok[:],
                        in1=srt[:, :, 0:1].to_broadcast([P, TT, E]),
                        op=mybir.AluOpType.subtract,
                    )
                    ex = routep.tile([P, TT, E], f32, tag="ex")
                    nc.scalar.activation(
                        ex[:], shif[:], mybir.ActivationFunctionType.Exp
                    )
                    mask = routep.tile([P, TT, E], f32, tag="mask")
                    nc.vector.tensor_tensor(
                        out=mask[:],
                        in0=s_tok[:],
                        in1=srt[:, :, K - 1 : K].to_broadcast([P, TT, E]),
                        op=mybir.AluOpType.is_ge,
                    )
                    nc.vector.tensor_tensor(
                        out=ex[:], in0=ex[:], in1=mask[:], op=mybir.AluOpType.mult
                    )
                    den = routep.tile([P, TT, 1], f32, tag="den")
                    nc.vector.reduce_sum(den[:], ex[:], axis=mybir.AxisListType.X)
                    rec = routep.tile([P, TT, 1], f32, tag="rec")
                    nc.vector.reciprocal(rec[:], den[:])
                    wtok = routep.tile([P, TT, E], f32, tag="wtok")
                    nc.vector.tensor_tensor(
                        out=wtok[:],
                        in0=ex[:],
                        in1=rec[:].to_broadcast([P, TT, E]),
                        op=mybir.AluOpType.mult,
                    )
                    # select this core's expert column (one-hot dot), token-major
                    wsel_g = routep.tile([P, TT, E], f32, tag="wselg")
                    nc.vector.tensor_tensor(
                        out=wsel_g[:],
                        in0=wtok[:],
                        in1=oh_sb[:].rearrange("p (t e) -> p t e", t=1).to_broadcast([P, TT, E]),
                        op=mybir.AluOpType.mult,
                    )
                    wsel = routep.tile([P, TT], f32, tag="wsel")
                    nc.vector.reduce_sum(
                        wsel[:].rearrange("p (t o) -> p t o", o=1),
                        wsel_g[:],
                        axis=mybir.AxisListType.X,
                    )
                    # transpose [P tok, TT] -> [TT, P]; flatten to a row; bcast
                    ps_w = psmisc.tile([TT, P], f32, tag="psm")
                    nc.tensor.transpose(ps_w[:], wsel[:], ident[:])
                    wrow4 = routep.tile([TT, P], f32, tag="wrow4")
                    nc.scalar.copy(wrow4[:], ps_w[:])
                    wrow = routep.tile([1, BLK], f32, tag="wrow")
                    nc.sync.dma_start(
                        wrow[:].rearrange("o (t p) -> o t p", t=TT), wrow4[:]
                    )
                    ps_b = psmisc.tile([P, BLK], f32, tag="psm")
                    nc.tensor.matmul(
                        ps_b[:], ones_row[:], wrow[:], start=True, stop=True
                    )
                    wbc = wbcp.tile([P, BLK], f32, tag="wbc")
                    nc.scalar.copy(wbc[:], ps_b[:])

                    # ---- FFN over this block ----
                    h_all = hpool.tile([P, FS, BLK], ffd, tag="h")
                    for fs in range(FS):
                        ps_h = psh.tile([P, BLK], f32, tag="psh")
                        for ds in range(DS):
                            nc.tensor.matmul(
                                ps_h[:],
                                w1_sb[:, ds, fs * P : (fs + 1) * P],
                                xbr[:, ds, :],
                                start=(ds == 0),
                                stop=(ds == DS - 1),
                            )
                        nc.scalar.activation(
                            h_all[:, fs, :], ps_h[:],
                            mybir.ActivationFunctionType.Relu,
                        )
                    for ds in range(DS):
                        ps_y = psy.tile([P, BLK], f32, tag="psy")
                        for fs in range(FS):
                            nc.tensor.matmul(
                                ps_y[:],
                                w2_sb[:, fs, ds * P : (ds + 1) * P],
                                h_all[:, fs, :],
                                start=(fs == 0),
                                stop=(fs == FS - 1),
                            )
                        yev = evpool.tile([P, BLK], f32, tag="yev")
                        nc.vector.tensor_tensor(
                            out=yev[:],
                            in0=ps_y[:],
                            in1=wbc[:],
                            op=mybir.AluOpType.mult,
                        )
                        nc.vector.scalar_tensor_tensor(
                            out=yev[:],
                            in0=xb[:, ds, :],
                            scalar=alpha_sb[:, 0:1],
                            in1=yev[:],
                            op0=mybir.AluOpType.mult,
                            op1=mybir.AluOpType.add,
                        )
                        nc.sync.dma_start(
                            ypart[half][ds * P : (ds + 1) * P, c0 : c0 + BLK],
                            yev[:],
                        )

                    if nb % HBLK == HBLK - 1:
                        nc.gpsimd.collective_compute(
                            "AllReduce",
                            mybir.AluOpType.add,
                            replica_groups=[list(range(E))],
                            ins=[ypart[half][:]],
                            outs=[ysum[half][:]],
                        )
                xsrc_halves = list(ysum)

            for h in range(nsplit):
                nc.sync.dma_start(
                    yout.ap()[:, h * NH : (h + 1) * NH], xsrc_halves[h][:]
                )
    return nc


_CACHE = {}


def _get_compiled():
    if "nc" not in _CACHE:
        nc = bacc.Bacc("TRN2", target_bir_lowering=False, debug=False, num_devices=8)
        build_moe(nc)
        nc.compile()
        _CACHE["nc"] = nc
    return _CACHE["nc"]


def kernel(x, protos, W1, W2, k):
    assert int(k) == 2
    B, S, Dx = x.shape
    L, E, D, F = W1.shape[0], W1.shape[1], W1.shape[2], W1.shape[3]
    N = B * S
    assert (B, S, Dx, L, E, D, F) == (2, 1024, 1024, 2, 8, 1024, 2048)

    nc = _get_compiled()

    xT = np.ascontiguousarray(np.asarray(x, dtype=np.float32).reshape(N, D).T)
    protT = np.ascontiguousarray(
        np.asarray(protos, dtype=np.float32).transpose(0, 2, 1)
    )
    W1 = np.asarray(W1, dtype=np.float32)
    W2 = np.asarray(W2, dtype=np.float32)

    in_maps = []
    for c in range(8):
        alpha = np.full((P, 1), 1.0 if c == 0 else 0.0, dtype=np.float32)
        oh = np.zeros((P, E), dtype=np.float32)
        oh[:, c] = 1.0
        in_maps.append(
            {
                "xT": xT,
                "prot": protT,
                "w1": np.ascontiguousarray(W1[:, c]),
                "w2": np.ascontiguousarray(W2[:, c]),
                "alpha": alpha,
                "onehot": oh,
            }
        )

    global _LAST_IN_MAPS
    _LAST_IN_MAPS = in_maps

    from concourse.bass_utils import run_bass_kernel_spmd

    res = run_bass_kernel_spmd(nc, in_maps, list(range(8)))
    out_T = res.results[0]["yout"]  # [D, N]
    return np.ascontiguousarray(out_T.T).reshape(B, S, D).astype(np.float32)

